# revision 18
# baseline (speedup 1.0000x reference)
"""Trainium2 Bass kernel for nn_BlockConv (PointNet-style GNN block), 8 cores.

Algebraic core: msg_e = concat(x_src, pos_src-pos_dst) @ W + b
  = A[src] - C[dst], with A = concat(x,pos)@W + b (per-node table) and
  C = pos@W[-3:] (per-dst, constant within a segment). segment_max over
  dst = (gather+max of A rows) - C[dst]. Pure memory problem.

Distribution: dst-sharded; each core computes the A-table rows for ITS
nodes only (slot layout, fp16) and TWO half-table AllGathers per conv
materialize the gather windows in shared HBM (halves = chunk ranges
s<SH / s>=SH of the slot layout; rows HALF..HALF+127 of every core's
half-slice are -BIG dummies for pass padding). The lo-side gather
passes depend only on half A, so they overlap half B's AllGather.
Skip path is computed in slot layout from a strided x read (no
regather); BN stats are chained PE matmuls (sum via ones vector,
sum-of-squares via v^T v diagonal), AllReduced across cores; BN1+ReLU
is one Activation op in transposed layout feeding conv2's matmuls.

Gather: dma_gather (int16 idx, round-robin over 4 SWDGE queues — on
real HW one queue serializes transfers); per window the core's nodes
are degree-sorted so pass k covers a slot prefix; DVE max chains
accumulate; one HBM round-trip re-gathers both accumulators in node
order and maxes them.
"""
import sys
import numpy as np

if "/opt/trn_rl_repo" not in sys.path:
    sys.path.insert(0, "/opt/trn_rl_repo")

BIG_NEG = -1.0e30
BIG_NEG16 = -60000.0
EPS = 1e-5

FULL_CFG = dict(N=50000, E=800000, CIN=64, COUT=128, NC=8, LO_LIM=32768)
MINI_CFG = dict(N=2048, E=16384, CIN=64, COUT=128, NC=8, LO_LIM=1024)
MID_CFG = dict(N=16384, E=262144, CIN=64, COUT=128, NC=8, LO_LIM=8192)


def _ceil(a, b):
    return (a + b - 1) // b


def _derived(cfg):
    N, NC = cfg["N"], cfg["NC"]
    NLOC = N // NC
    SLOC = _ceil(NLOC, 128)       # smallest divisor of NLOC >= ceil(NLOC/128)
    while NLOC % SLOC:
        SLOC += 1
    NSLOT = SLOC * 128
    assert SLOC % 2 == 0
    SH = SLOC // 2                # chunks per table half (A: s<SH, B: s>=SH)
    HALF = SH * 128
    HALFP = HALF + 128            # +128 dummy -BIG rows per core half-slice
    RTOT = NC * HALFP             # rows per window (one window per half)
    PV = min(128, NLOC // SLOC)   # valid partitions (slots p*SLOC+s < NLOC)
    return NLOC, SLOC, NSLOT, SH, HALFP, RTOT, PV


def _wrap16(ids):
    """flat int list (len % 128 == 0) -> [128, len//16] int16 wrapped:
    unwrapped[j] = g[j%16, j//16], replicated over the 8 core groups."""
    a = np.asarray(ids, np.int64)
    assert a.size % 128 == 0 and a.min() >= 0 and a.max() < 32768
    g = a.reshape(a.size // 16, 16).T.astype(np.int16)   # [16, L/16]
    return np.tile(g, (8, 1))                            # [128, L/16]


def host_prep(edge_index, pos, cfg):
    N, NC, LO_LIM = cfg["N"], cfg["NC"], cfg["LO_LIM"]
    NLOC, SLOC, NSLOT, SH, HALFP, RTOT, PV = _derived(cfg)
    assert RTOT <= 32768
    src = np.asarray(edge_index[0], np.int64)
    dst = np.asarray(edge_index[1], np.int64)
    nl = src % NLOC
    schunk = nl % SLOC            # chunk index within the slot layout
    # half A rows: s<SH at row c*HALFP + p*SH + s; half B: s-SH
    rows = ((src // NLOC) * HALFP + (nl // SLOC) * SH
            + np.where(schunk < SH, schunk, schunk - SH))
    core_of = dst // NLOC
    dummy_lo = HALF = SH * 128                      # core 0 dummy block
    dummy_hi = (NC - 1) * HALFP + HALF              # last core dummy block

    sides = [[], []]     # sides[0][c] = half-A side of core c
    for c in range(NC):
        m = core_of == c
        s_rows = rows[m]
        s_half = schunk[m] >= SH
        d_loc = dst[m] - c * NLOC
        for si, sel in ((0, ~s_half), (1, s_half)):
            s = s_rows[sel]
            d = d_loc[sel]
            deg = np.bincount(d, minlength=NSLOT)
            order = np.argsort(-deg, kind="stable")
            slot_of = np.empty(NSLOT, np.int64)
            slot_of[order] = np.arange(NSLOT)
            isort = np.argsort(d, kind="stable")
            starts = np.zeros(NSLOT + 1, np.int64)
            np.cumsum(deg, out=starts[1:])
            sides[si].append({"deg": deg, "order": order, "slot_of": slot_of,
                              "s_sorted": s[isort], "starts": starts,
                              "cnts": np.sort(deg)[::-1]})

    sched = []
    for si in range(2):
        Sk = []
        kmax = max(int(sd["cnts"][0]) for sd in sides[si])
        for k in range(kmax):
            cnt = max(int((sd["cnts"] > k).sum()) for sd in sides[si])
            if cnt == 0:
                break
            Sk.append(_ceil(cnt, 128))
        sched.append(Sk)

    j = np.arange(NSLOT)
    n_of_j = (j % 128) * SLOC + j // 128

    per_core = []
    for c in range(NC):
        blocks = {0: [], 1: []}
        for si in range(2):
            sd = sides[si][c]
            dummy = dummy_lo if si == 0 else dummy_hi
            for k, S in enumerate(sched[si]):
                L = S * 128
                ids = np.full(L, dummy, np.int64)
                nsl = int((sd["cnts"] > k).sum())
                nodes = sd["order"][:nsl]
                ids[:nsl] = sd["s_sorted"][sd["starts"][nodes] + k]
                blocks[si].append(_wrap16(ids))
        gi_lo = (np.concatenate(blocks[0], axis=1) if blocks[0]
                 else np.zeros((128, 8), np.int16))
        gi_hi = (np.concatenate(blocks[1], axis=1) if blocks[1]
                 else np.zeros((128, 8), np.int16))
        mg_lo = _wrap16(sides[0][c]["slot_of"][n_of_j])
        mg_hi = _wrap16(sides[1][c]["slot_of"][n_of_j] + NSLOT)
        gnode = np.minimum(c * NLOC + n_of_j, N - 1)
        posm = np.ascontiguousarray(
            np.asarray(pos)[gnode].T).astype(np.float16)
        per_core.append({"gi_lo": gi_lo, "gi_hi": gi_hi, "mg_lo": mg_lo,
                         "mg_hi": mg_hi, "posm": posm})
    return per_core, sched


def build_bass(cfg, sched, reps=1, timeline=False, no_cc=False):
    import concourse.bass as bass
    import concourse.bacc as bacc
    import concourse.tile as tile
    from concourse import mybir
    from concourse.masks import make_identity
    import contextlib

    N, NC = cfg["N"], cfg["NC"]
    CIN, COUT = cfg["CIN"], cfg["COUT"]
    LO_LIM = cfg["LO_LIM"]
    NLOC, SLOC, NSLOT, SH, HALFP, RTOT, PV = _derived(cfg)
    HALF = SH * 128
    f32, bf16, i16 = mybir.dt.float32, mybir.dt.float16, mybir.dt.int16
    OP = mybir.AluOpType
    AF = mybir.ActivationFunctionType

    nc = bacc.Bacc(num_devices=(1 if timeline else NC), name="blockconv",
                   dynamic_dma_scratch_size=16384, num_swdge_queues=4)

    xs_in = nc.dram_tensor("xs", [NLOC, CIN], f32, kind="ExternalInput")
    posm_in = nc.dram_tensor("posm", [3, NSLOT], bf16, kind="ExternalInput")
    wt = {}
    for nm, shp in (("W1", [CIN + 3, COUT]), ("b1", [1, COUT]),
                    ("W2", [COUT + 3, COUT]), ("b2", [1, COUT]),
                    ("Wl", [CIN, COUT]), ("bl", [1, COUT]),
                    ("g1", [COUT, 1]), ("be1", [COUT, 1]), ("g2", [COUT, 1]),
                    ("be2", [COUT, 1]), ("gl", [COUT, 1]), ("bel", [COUT, 1])):
        wt[nm] = nc.dram_tensor(nm, shp, f32, kind="ExternalInput")

    Wlo = max(sum(sched[0]), 1) * 8
    Whi = max(sum(sched[1]), 1) * 8
    gi_lo_in = nc.dram_tensor("gi_lo", [128, Wlo], i16, kind="ExternalInput")
    gi_hi_in = nc.dram_tensor("gi_hi", [128, Whi], i16, kind="ExternalInput")
    mg_lo_in = nc.dram_tensor("mg_lo", [128, NSLOT // 16], i16, kind="ExternalInput")
    mg_hi_in = nc.dram_tensor("mg_hi", [128, NSLOT // 16], i16, kind="ExternalInput")

    out_t = nc.dram_tensor("out", [NSLOT, COUT], f32, kind="ExternalOutput")

    shared = "Local" if timeline else "Shared"
    agi = {}
    ago = {}
    for cv in (1, 2):
        for hf in "ab":
            agi[cv, hf] = nc.dram_tensor(f"ag{cv}{hf}_in", [HALFP, COUT], bf16)
            ago[cv, hf] = nc.dram_tensor(f"ag{cv}{hf}_out", [NC, HALFP, COUT],
                                         bf16, addr_space=shared)
    mbuf = nc.dram_tensor("mbuf", [2 * NSLOT, COUT], bf16)
    ar_i = nc.dram_tensor("ar_in", [COUT, 4], f32)
    ar_o = nc.dram_tensor("ar_out", [COUT, 4], f32, addr_space=shared)
    rowbuf = nc.dram_tensor("rowbuf", [6, COUT], f32)
    ar2_i = nc.dram_tensor("ar2_in", [COUT, 2], f32)
    ar2_o = nc.dram_tensor("ar2_out", [COUT, 2], f32, addr_space=shared)
    groups = [list(range(NC))]

    with tile.TileContext(nc) as tc:
        ctx = contextlib.ExitStack()
        with ctx:
            sing = ctx.enter_context(tc.tile_pool(name="sing", bufs=1))
            xp = ctx.enter_context(tc.tile_pool(name="xp", bufs=3))
            pp = ctx.enter_context(tc.tile_pool(name="pp", bufs=2, space="PSUM"))
            pq = ctx.enter_context(tc.tile_pool(name="pq", bufs=2, space="PSUM"))
            pr = ctx.enter_context(tc.tile_pool(name="pr", bufs=1, space="PSUM"))
            cp = ctx.enter_context(tc.tile_pool(name="cp", bufs=4))
            ap_ = ctx.enter_context(tc.tile_pool(name="ap", bufs=1))
            st = ctx.enter_context(tc.tile_pool(name="st", bufs=2))
            sm = ctx.enter_context(tc.tile_pool(name="sm", bufs=2))

            ident = sing.tile([128, 128], f32)
            make_identity(nc, ident)
            identb = sing.tile([128, 128], bf16)
            nc.vector.tensor_copy(out=identb[:], in_=ident[:])
            ones1 = sing.tile([1, 128], f32)
            nc.vector.memset(ones1[:], 1.0)
            ones1v = sing.tile([1, 128], f32)   # valid-partition row mask
            nc.vector.memset(ones1v[:], 1.0)
            if PV < 128:
                nc.vector.memset(ones1v[:, PV:128], 0.0)
            onesp = sing.tile([128, 1], f32)
            nc.vector.memset(onesp[:], 1.0)
            onespb = sing.tile([128, 1], bf16)
            nc.vector.memset(onespb[:], 1.0)
            negbig = sing.tile([128, COUT], bf16)
            nc.vector.memset(negbig[:], BIG_NEG16)
            epsv = sing.tile([COUT, 1], f32)
            nc.vector.memset(epsv[:], EPS)

            W1s = sing.tile([CIN + 3, COUT], f32)
            nc.sync.dma_start(W1s[:], wt["W1"][:])
            W1pb = sing.tile([3, COUT], bf16)
            nc.vector.tensor_copy(out=W1pb[:], in_=W1s[CIN:CIN + 3, :])
            W2af = sing.tile([COUT, COUT], f32)
            nc.sync.dma_start(W2af[:], wt["W2"][0:COUT, :])
            W2ab = sing.tile([COUT, COUT], bf16)
            nc.vector.tensor_copy(out=W2ab[:], in_=W2af[:])
            W2pf = sing.tile([3, COUT], f32)
            nc.sync.dma_start(W2pf[:], wt["W2"][COUT:COUT + 3, :])
            W2pb = sing.tile([3, COUT], bf16)
            nc.vector.tensor_copy(out=W2pb[:], in_=W2pf[:])
            Wlf = sing.tile([CIN, COUT], f32)
            nc.sync.dma_start(Wlf[:], wt["Wl"][:])
            brow = {}
            for nm in ("b1", "b2", "bl"):
                t = sing.tile([1, COUT], f32, tag=f"br_{nm}")
                nc.sync.dma_start(t[:], wt[nm][:])
                brow[nm] = t
            pvec = {}
            for nm in ("g1", "be1", "g2", "be2", "gl", "bel"):
                v = sing.tile([COUT, 1], f32, tag=f"pv_{nm}")
                nc.sync.dma_start(v[:], wt[nm][:])
                pvec[nm] = v

            posmb = sing.tile([3, NSLOT], bf16)
            nc.sync.dma_start(posmb[:], posm_in[:])
            idx_lo = sing.tile([128, Wlo], i16)
            nc.sync.dma_start(idx_lo[:], gi_lo_in[:])
            idx_hi = sing.tile([128, Whi], i16)
            nc.sync.dma_start(idx_hi[:], gi_hi_in[:])
            midx = {}
            for nm, t_ in (("lo", mg_lo_in), ("hi", mg_hi_in)):
                m_ = sing.tile([128, NSLOT // 16], i16, tag=f"mi_{nm}")
                nc.sync.dma_start(m_[:], t_[:])
                midx[nm] = m_

            for _rep in range(reps):
                # -BIG dummy rows HALF..HALF+127 of this core's half-slices
                for cv in (1, 2):
                    for hf in "ab":
                        nc.sync.dma_start(agi[cv, hf][HALF:HALFP, :], negbig[:])

                # ---------- build a1 + skip in slot layout -------------
                xbig = sing.tile([128, SLOC, CIN], f32, tag="xbig")
                nc.vector.memset(xbig[:], 0.0)
                nc.sync.dma_start(
                    xbig[0:PV], xs_in[:].rearrange("(p s) c -> p s c", s=SLOC))
                a1 = ap_.tile([128, SLOC, COUT], bf16, tag="big1")
                skt = ap_.tile([128, SLOC, COUT], f32, tag="bigsk")
                for s in range(SLOC):
                    ps = pp.tile([128, 128], f32, tag="pst")
                    nc.tensor.transpose(out=ps[0:CIN, :], in_=xbig[:, s, :], identity=ident[:])
                    xT = xp.tile([CIN, 128], f32, tag="xT")
                    nc.scalar.copy(out=xT[:], in_=ps[0:CIN, :])
                    pb = pq.tile([128, COUT], f32, tag="pout")
                    nc.tensor.matmul(out=pb[:], lhsT=xT[:], rhs=W1s[0:CIN, :], start=True, stop=False)
                    nc.tensor.matmul(out=pb[:], lhsT=posmb[:, s * 128:(s + 1) * 128], rhs=W1pb[:], start=False, stop=False)
                    nc.tensor.matmul(out=pb[:], lhsT=ones1[:], rhs=brow["b1"][:], start=False, stop=True)
                    nc.vector.tensor_copy(out=a1[:, s, :], in_=pb[:])
                    pl = pq.tile([128, COUT], f32, tag="pout")
                    nc.tensor.matmul(out=pl[:], lhsT=xT[:], rhs=Wlf[:], start=True, stop=False)
                    nc.tensor.matmul(out=pl[:], lhsT=ones1v[:], rhs=brow["bl"][:], start=False, stop=True)
                    nc.scalar.copy(out=skt[:, s, :], in_=pl[:])

                arst = sing.tile([COUT, 4], f32)

                # PE-chained stats: sums via ones matmul, squares via diag(v^T v)
                def stats_into(vtile, arcols, ones_vec):
                    psum_ = pr.tile([COUT, 1], f32, tag="psum")
                    psq = pr.tile([128, 128], f32, tag="psq")
                    for s in range(SLOC):
                        nc.tensor.matmul(out=psum_[:], lhsT=vtile[:, s, :], rhs=ones_vec[:],
                                         start=(s == 0), stop=(s == SLOC - 1))
                        nc.tensor.matmul(out=psq[:], lhsT=vtile[:, s, :], rhs=vtile[:, s, :],
                                         start=(s == 0), stop=(s == SLOC - 1))
                    sq = sm.tile([128, 128], f32, tag="sqd")
                    nc.vector.tensor_tensor(out=sq[:], in0=psq[:], in1=ident[:], op=OP.mult)
                    nc.vector.tensor_copy(out=arcols[:, 0:1], in_=psum_[:])
                    nc.vector.tensor_reduce(out=arcols[:, 1:2], in_=sq[:],
                                            op=OP.add, axis=mybir.AxisListType.X)

                stats_into(skt, arst[:, 2:4], onesp)

                def table_write(agpair, atile):
                    nc.sync.dma_start(
                        agpair[0][0:PV * SH, :].rearrange("(p s) f -> p s f", s=SH),
                        atile[0:PV, 0:SH, :])
                    nc.sync.dma_start(
                        agpair[1][0:PV * SH, :].rearrange("(p s) f -> p s f", s=SH),
                        atile[0:PV, SH:SLOC, :])

                table_write((agi[1, "a"], agi[1, "b"]), a1)

                def allgather(src, dst):
                    if no_cc:
                        return
                    if timeline:
                        for q in range(_ceil(HALFP, 128)):
                            r0, r1 = q * 128, min((q + 1) * 128, HALFP)
                            t_ = cp.tile([128, COUT], bf16, tag="agb")
                            nc.sync.dma_start(t_[0:r1 - r0], src[r0:r1, :])
                            nc.sync.dma_start(dst[0, r0:r1, :], t_[0:r1 - r0])
                    else:
                        nc.gpsimd.collective_compute(
                            "AllGather", OP.bypass, replica_groups=groups,
                            ins=[src[:]], outs=[dst[:]])

                allgather(agi[1, "a"], ago[1, "a"])
                allgather(agi[1, "b"], ago[1, "b"])

                # ---------------- gather-max passes ----------------
                GMAX = 8   # max 8*128=1024 indices per dma_gather (SWDGE ring cap)

                qrr = [0]   # round-robin over the 4 SWDGE queues

                def gather_chunked(dst3, in_ap, idxt, chunk0, nchunks):
                    a = 0
                    while a < nchunks:
                        b = min(a + GMAX, nchunks)
                        nc.gpsimd.dma_gather(
                            out_ap=dst3[:, a:b, :], in_ap=in_ap,
                            idxs_ap=idxt[:, (chunk0 + a) * 8:(chunk0 + b) * 8],
                            num_idxs=(b - a) * 128, num_idxs_reg=(b - a) * 128,
                            elem_size=COUT, queue_num=qrr[0] % 4)
                        qrr[0] += 1
                        a = b

                def gather_conv(cv, acc_tags):
                    flat_a = ago[cv, "a"][:].rearrange("c n f -> (c n) f")
                    flat_b = ago[cv, "b"][:].rearrange("c n f -> (c n) f")
                    acc = {}
                    for snm, tg in zip(("lo", "hi"), acc_tags):
                        a = ap_.tile([128, SLOC, COUT], bf16, tag=tg)
                        nc.gpsimd.memset(a[:], BIG_NEG16)
                        acc[snm] = a
                    for snm, idxt, flat in (("lo", idx_lo, flat_a),
                                            ("hi", idx_hi, flat_b)):
                        off = 0
                        for k, S in enumerate(sched[0 if snm == "lo" else 1]):
                            stg = st.tile([128, SLOC, COUT], bf16, tag="stage")
                            gather_chunked(stg[:, 0:S, :], flat[:],
                                           idxt, off // 8, S)
                            nc.vector.tensor_tensor(
                                out=acc[snm][:, 0:S, :], in0=acc[snm][:, 0:S, :],
                                in1=stg[:, 0:S, :], op=OP.max)
                            off += 8 * S
                    # merge via HBM round-trip, node order
                    nc.sync.dma_start(
                        mbuf[0:NSLOT, :].rearrange("(s p) f -> p s f", p=128), acc["lo"][:])
                    nc.sync.dma_start(
                        mbuf[NSLOT:2 * NSLOT, :].rearrange("(s p) f -> p s f", p=128), acc["hi"][:])
                    g1t = st.tile([128, SLOC, COUT], bf16, tag="stage")
                    gather_chunked(g1t[:], mbuf[:], midx["lo"], 0, SLOC)
                    g2t = st.tile([128, SLOC, COUT], bf16, tag="stage")
                    gather_chunked(g2t[:], mbuf[:], midx["hi"], 0, SLOC)
                    agg = ap_.tile([128, SLOC, COUT], bf16, tag=acc_tags[0])
                    nc.vector.tensor_tensor(out=agg[:], in0=g1t[:], in1=g2t[:], op=OP.max)
                    return agg

                agg1 = gather_conv(1, ("big1", "big2"))

                # v = (agg - pc) * (agg > -1e29), per chunk s
                def v_compute(agg, Wp, vtag, vT=None):
                    v = ap_.tile([128, SLOC, COUT], bf16, tag=vtag)
                    for s in range(SLOC):
                        pc = pq.tile([128, COUT], f32, tag="pout")
                        nc.tensor.matmul(out=pc[:], lhsT=posmb[:, s * 128:(s + 1) * 128],
                                         rhs=Wp[:], start=True, stop=True)
                        msk = cp.tile([128, COUT], bf16, tag="msk")
                        nc.vector.tensor_scalar(out=msk[:], in0=agg[:, s, :], scalar1=-30000.0,
                                                scalar2=None, op0=OP.is_gt)
                        t_ = cp.tile([128, COUT], bf16, tag="tv")
                        nc.vector.tensor_tensor(out=t_[:], in0=agg[:, s, :], in1=pc[:], op=OP.subtract)
                        nc.vector.tensor_tensor(out=v[:, s, :], in0=t_[:], in1=msk[:], op=OP.mult)
                        if vT is not None:
                            pt = pr.tile([128, 128], bf16, tag="psb")
                            nc.tensor.transpose(out=pt[:], in_=v[:, s, :], identity=identb[:])
                            nc.scalar.copy(out=vT[:, s, :], in_=pt[:])
                    return v

                v1T = ap_.tile([128, SLOC, 128], bf16, tag="big3")
                v1 = v_compute(agg1, W1pb, "big2", vT=v1T)
                stats_into(v1, arst[:, 0:2], onespb)
                nc.sync.dma_start(ar_i[:], arst[:])
                if no_cc:
                    pass
                elif timeline:
                    _t = sm.tile([COUT, 4], f32, tag="cc1")
                    nc.sync.dma_start(_t[:], ar_i[:])
                    nc.sync.dma_start(ar_o[:], _t[:])
                else:
                    nc.gpsimd.collective_compute("AllReduce", OP.add, replica_groups=groups,
                                                 ins=[ar_i[:]], outs=[ar_o[:]])
                arres = sing.tile([COUT, 4], f32, tag="arres")
                nc.sync.dma_start(arres[:], ar_o[:])

                # BN params: scale = g * rsqrt(var+eps), shift = be - mean*scale
                def bn_vecs(sum_ap, sq_ap, g_v, be_v, tagp):
                    mean = sm.tile([COUT, 1], f32, tag=f"{tagp}_m")
                    nc.vector.tensor_scalar(out=mean[:], in0=sum_ap, scalar1=1.0 / N,
                                            scalar2=None, op0=OP.mult)
                    ex2 = sm.tile([COUT, 1], f32, tag=f"{tagp}_e")
                    nc.vector.tensor_scalar(out=ex2[:], in0=sq_ap, scalar1=1.0 / N,
                                            scalar2=None, op0=OP.mult)
                    m2 = sm.tile([COUT, 1], f32, tag=f"{tagp}_m2")
                    nc.vector.tensor_tensor(out=m2[:], in0=mean[:], in1=mean[:], op=OP.mult)
                    var = sm.tile([COUT, 1], f32, tag=f"{tagp}_v")
                    nc.vector.tensor_tensor(out=var[:], in0=ex2[:], in1=m2[:], op=OP.subtract)
                    sd = sm.tile([COUT, 1], f32, tag=f"{tagp}_sd")
                    nc.scalar.activation(out=sd[:], in_=var[:], func=AF.Sqrt, bias=epsv[:], scale=1.0)
                    rstd = sm.tile([COUT, 1], f32, tag=f"{tagp}_r")
                    nc.vector.reciprocal(out=rstd[:], in_=sd[:])
                    ssh = sm.tile([COUT, 2], f32, tag=f"{tagp}_ssh")
                    nc.vector.tensor_tensor(out=ssh[:, 0:1], in0=rstd[:], in1=g_v[:], op=OP.mult)
                    ms = sm.tile([COUT, 1], f32, tag=f"{tagp}_ms")
                    nc.vector.tensor_tensor(out=ms[:], in0=mean[:], in1=ssh[:, 0:1], op=OP.mult)
                    nc.vector.tensor_tensor(out=ssh[:, 1:2], in0=be_v[:], in1=ms[:], op=OP.subtract)
                    return ssh

                def bn_rows(ssh, tagp):
                    # transpose [COUT,2] -> [2, COUT] rows (scale row 0, shift row 1)
                    prow = pr.tile([2, COUT], f32, tag="prow")
                    nc.tensor.transpose(out=prow[:], in_=ssh[:], identity=ident[:])
                    rows = sing.tile([2, COUT], f32, tag=f"{tagp}_rows")
                    nc.vector.tensor_copy(out=rows[:], in_=prow[:])
                    slot = {"bn1": 0, "bnl": 2, "bn2": 4}[tagp]
                    nc.sync.dma_start(rowbuf[slot:slot + 2, :], rows[:])
                    bc = sing.tile([128, 2, COUT], f32, tag=f"{tagp}_bc")
                    rap = rowbuf[slot:slot + 2, :]
                    nc.sync.dma_start(bc[:], bass.AP(tensor=rap.tensor, offset=rap.offset,
                                                     ap=[[0, 128]] + list(rap.ap)))
                    return bc

                ssh1 = bn_vecs(arres[:, 0:1], arres[:, 1:2], pvec["g1"], pvec["be1"], "bn1")
                sshl = bn_vecs(arres[:, 2:3], arres[:, 3:4], pvec["gl"], pvec["bel"], "bnl")
                rowsl = bn_rows(sshl, "bnl")

                # h1T = relu(v1T*scale1 + shift1): one Act op in transposed
                # (feature-major) layout, using the v1T built during the AR
                nc.scalar.activation(out=v1T[:].rearrange("f s n -> f (s n)"),
                                     in_=v1T[:].rearrange("f s n -> f (s n)"),
                                     func=AF.Relu, bias=ssh1[:, 1:2], scale=ssh1[:, 0:1])

                # ---------- a2 = [h1|pos]@W2 + b2 (slot layout) --------
                a2 = ap_.tile([128, SLOC, COUT], bf16, tag="big1")
                for s in range(SLOC):
                    pb = pq.tile([128, COUT], f32, tag="pout")
                    nc.tensor.matmul(out=pb[:], lhsT=v1T[:, s, :], rhs=W2ab[:], start=True, stop=False)
                    nc.tensor.matmul(out=pb[:], lhsT=posmb[:, s * 128:(s + 1) * 128], rhs=W2pb[:], start=False, stop=False)
                    nc.tensor.matmul(out=pb[:], lhsT=ones1[:], rhs=brow["b2"][:], start=False, stop=True)
                    nc.vector.tensor_copy(out=a2[:, s, :], in_=pb[:])
                table_write((agi[2, "a"], agi[2, "b"]), a2)
                allgather(agi[2, "a"], ago[2, "a"])
                allgather(agi[2, "b"], ago[2, "b"])

                agg2 = gather_conv(2, ("big1", "big2"))
                v2 = v_compute(agg2, W2pb, "big2")

                arst2 = sing.tile([COUT, 2], f32, tag="arst2")
                stats_into(v2, arst2[:], onespb)
                nc.sync.dma_start(ar2_i[:], arst2[:])
                if no_cc:
                    pass
                elif timeline:
                    _t = sm.tile([COUT, 2], f32, tag="cc2")
                    nc.sync.dma_start(_t[:], ar2_i[:])
                    nc.sync.dma_start(ar2_o[:], _t[:])
                else:
                    nc.gpsimd.collective_compute("AllReduce", OP.add, replica_groups=groups,
                                                 ins=[ar2_i[:]], outs=[ar2_o[:]])
                arres2 = sing.tile([COUT, 2], f32, tag="arres2")
                nc.sync.dma_start(arres2[:], ar2_o[:])
                rows2 = bn_rows(bn_vecs(arres2[:, 0:1], arres2[:, 1:2], pvec["g2"], pvec["be2"], "bn2"), "bn2")

                # final = relu(bn2(v2) + bnl(skip)); skt transformed in place
                fin = ap_.tile([128, SLOC, COUT], f32, tag="bigf")
                nc.vector.tensor_tensor(out=fin[:], in0=v2[:],
                                        in1=rows2[:, 0:1, :].to_broadcast([128, SLOC, COUT]), op=OP.mult)
                nc.vector.tensor_tensor(out=fin[:], in0=fin[:],
                                        in1=rows2[:, 1:2, :].to_broadcast([128, SLOC, COUT]), op=OP.add)
                nc.vector.tensor_tensor(out=skt[:], in0=skt[:],
                                        in1=rowsl[:, 0:1, :].to_broadcast([128, SLOC, COUT]), op=OP.mult)
                nc.vector.tensor_tensor(out=skt[:], in0=skt[:],
                                        in1=rowsl[:, 1:2, :].to_broadcast([128, SLOC, COUT]), op=OP.add)
                nc.vector.tensor_tensor(out=fin[:], in0=fin[:], in1=skt[:], op=OP.add)
                nc.vector.tensor_scalar(out=fin[:], in0=fin[:], scalar1=0.0, scalar2=None, op0=OP.max)
                nc.sync.dma_start(out_t[:].rearrange("(p s) f -> p s f", p=128), fin[:])

    nc.compile()
    return nc


def make_in_maps(inputs, cfg, per_core):
    N, NC, CIN = cfg["N"], cfg["NC"], cfg["CIN"]
    NLOC = N // NC
    x = np.ascontiguousarray(np.asarray(inputs["x"], np.float32))
    shared = dict(
        W1=np.asarray(inputs["W1"], np.float32),
        b1=np.asarray(inputs["b1"], np.float32).reshape(1, -1),
        W2=np.asarray(inputs["W2"], np.float32),
        b2=np.asarray(inputs["b2"], np.float32).reshape(1, -1),
        Wl=np.asarray(inputs["Wl"], np.float32),
        bl=np.asarray(inputs["bl"], np.float32).reshape(1, -1),
        g1=np.asarray(inputs["g1"], np.float32).reshape(-1, 1),
        be1=np.asarray(inputs["be1"], np.float32).reshape(-1, 1),
        g2=np.asarray(inputs["g2"], np.float32).reshape(-1, 1),
        be2=np.asarray(inputs["be2"], np.float32).reshape(-1, 1),
        gl=np.asarray(inputs["gl"], np.float32).reshape(-1, 1),
        bel=np.asarray(inputs["bel"], np.float32).reshape(-1, 1),
    )
    in_maps = []
    for c in range(NC):
        m = dict(shared)
        m["xs"] = np.ascontiguousarray(x[c * NLOC:(c + 1) * NLOC])
        pc = per_core[c]
        for k in ("gi_lo", "gi_hi", "mg_lo", "mg_hi", "posm"):
            m[k] = pc[k]
        in_maps.append(m)
    return in_maps


_CACHE = {}


def run(inputs, cfg, use_sim=False, trace=False):
    per_core, sched = host_prep(inputs["edge_index"], inputs["pos"], cfg)
    key = (cfg["N"], tuple(sched[0]), tuple(sched[1]))
    if key not in _CACHE:
        _CACHE[key] = build_bass(cfg, sched)
    nc = _CACHE[key]
    in_maps = make_in_maps(inputs, cfg, per_core)
    NC = cfg["NC"]
    NLOC = cfg["N"] // NC
    if use_sim:
        from concourse.bass_interp import MultiCoreSim
        sim = MultiCoreSim(nc, num_cores=NC, require_finite=False, require_nnan=False)
        for c in range(NC):
            for k, v in in_maps[c].items():
                sim.cores[c].tensor(k)[:] = v
        sim.simulate(check_with_hw=False)
        outs = [np.array(sim.cores[c].tensor("out")) for c in range(NC)]
        res = None
    else:
        from concourse.bass_utils import run_bass_kernel_spmd
        res = run_bass_kernel_spmd(nc, in_maps, core_ids=list(range(NC)), trace=trace)
        outs = [res.results[c]["out"] for c in range(NC)]
    full = np.concatenate([o[:NLOC] for o in outs], axis=0)
    return full, res


def kernel(**inputs):
    out, _ = run(inputs, FULL_CFG, use_sim=False)
    return out


# revision 20
# speedup vs baseline: 1.8569x; 1.8569x over previous
"""Trainium2 Bass kernel for nn_BlockConv (PointNet-style GNN block), 8 cores.

Algebraic core: msg_e = concat(x_src, pos_src-pos_dst) @ W + b
  = A[src] - C[dst], with A = concat(x,pos)@W + b (per-node table) and
  C = pos@W[-3:] (per-dst, constant within a segment). segment_max over
  dst = (gather+max of A rows) - C[dst]. Pure memory problem.

Distribution: dst-sharded; each core computes the A-table rows for ITS
nodes only (slot layout, fp16) and TWO half-table AllGathers per conv
materialize the gather windows in shared HBM (halves = chunk ranges
s<SH / s>=SH of the slot layout; rows HALF..HALF+127 of every core's
half-slice are -BIG dummies for pass padding). The lo-side gather
passes depend only on half A, so they overlap half B's AllGather.
Skip path is computed in slot layout from a strided x read (no
regather); BN stats are chained PE matmuls (sum via ones vector,
sum-of-squares via v^T v diagonal), AllReduced across cores; BN1+ReLU
is one Activation op in transposed layout feeding conv2's matmuls.

Gather: dma_gather (int16 idx, round-robin over 4 SWDGE queues — on
real HW one queue serializes transfers); per window the core's nodes
are degree-sorted so pass k covers a slot prefix; DVE max chains
accumulate; one HBM round-trip re-gathers both accumulators in node
order and maxes them.
"""
import sys
import numpy as np

if "/opt/trn_rl_repo" not in sys.path:
    sys.path.insert(0, "/opt/trn_rl_repo")

BIG_NEG = -1.0e30
BIG_NEG16 = -60000.0
EPS = 1e-5

FULL_CFG = dict(N=50000, E=800000, CIN=64, COUT=128, NC=8, LO_LIM=32768)
MINI_CFG = dict(N=2048, E=16384, CIN=64, COUT=128, NC=8, LO_LIM=1024)
MID_CFG = dict(N=16384, E=262144, CIN=64, COUT=128, NC=8, LO_LIM=8192)


def _ceil(a, b):
    return (a + b - 1) // b


def _derived(cfg):
    N, NC = cfg["N"], cfg["NC"]
    NLOC = N // NC
    SLOC = _ceil(NLOC, 128)       # smallest divisor of NLOC >= ceil(NLOC/128)
    while NLOC % SLOC:
        SLOC += 1
    NSLOT = SLOC * 128
    assert SLOC % 2 == 0
    SH = SLOC // 2                # chunks per table half (A: s<SH, B: s>=SH)
    HALF = SH * 128
    HALFP = HALF + 128            # +128 dummy -BIG rows per core half-slice
    RTOT = NC * HALFP             # rows per window (one window per half)
    PV = min(128, NLOC // SLOC)   # valid partitions (slots p*SLOC+s < NLOC)
    return NLOC, SLOC, NSLOT, SH, HALFP, RTOT, PV


def _wrap16(ids):
    """flat int list (len % 128 == 0) -> [128, len//16] int16 wrapped:
    unwrapped[j] = g[j%16, j//16], replicated over the 8 core groups."""
    a = np.asarray(ids, np.int64)
    assert a.size % 128 == 0 and a.min() >= 0 and a.max() < 32768
    g = a.reshape(a.size // 16, 16).T.astype(np.int16)   # [16, L/16]
    return np.tile(g, (8, 1))                            # [128, L/16]


def host_prep(edge_index, pos, cfg):
    N, NC, LO_LIM = cfg["N"], cfg["NC"], cfg["LO_LIM"]
    NLOC, SLOC, NSLOT, SH, HALFP, RTOT, PV = _derived(cfg)
    assert RTOT <= 32768
    src = np.asarray(edge_index[0], np.int64)
    dst = np.asarray(edge_index[1], np.int64)
    nl = src % NLOC
    schunk = nl % SLOC            # chunk index within the slot layout
    # half A rows: s<SH at row c*HALFP + p*SH + s; half B: s-SH
    rows = ((src // NLOC) * HALFP + (nl // SLOC) * SH
            + np.where(schunk < SH, schunk, schunk - SH))
    core_of = dst // NLOC
    dummy_lo = HALF = SH * 128                      # core 0 dummy block
    dummy_hi = (NC - 1) * HALFP + HALF              # last core dummy block

    sides = [[], []]     # sides[0][c] = half-A side of core c
    for c in range(NC):
        m = core_of == c
        s_rows = rows[m]
        s_half = schunk[m] >= SH
        d_loc = dst[m] - c * NLOC
        for si, sel in ((0, ~s_half), (1, s_half)):
            s = s_rows[sel]
            d = d_loc[sel]
            deg = np.bincount(d, minlength=NSLOT)
            order = np.argsort(-deg, kind="stable")
            slot_of = np.empty(NSLOT, np.int64)
            slot_of[order] = np.arange(NSLOT)
            isort = np.argsort(d, kind="stable")
            starts = np.zeros(NSLOT + 1, np.int64)
            np.cumsum(deg, out=starts[1:])
            sides[si].append({"deg": deg, "order": order, "slot_of": slot_of,
                              "s_sorted": s[isort], "starts": starts,
                              "cnts": np.sort(deg)[::-1]})

    sched = []
    for si in range(2):
        Sk = []
        kmax = max(int(sd["cnts"][0]) for sd in sides[si])
        for k in range(kmax):
            cnt = max(int((sd["cnts"] > k).sum()) for sd in sides[si])
            if cnt == 0:
                break
            Sk.append(_ceil(cnt, 128))
        sched.append(Sk)

    j = np.arange(NSLOT)
    n_of_j = (j % 128) * SLOC + j // 128

    per_core = []
    for c in range(NC):
        blocks = {0: [], 1: []}
        for si in range(2):
            sd = sides[si][c]
            dummy = dummy_lo if si == 0 else dummy_hi
            for k, S in enumerate(sched[si]):
                L = S * 128
                ids = np.full(L, dummy, np.int64)
                nsl = int((sd["cnts"] > k).sum())
                nodes = sd["order"][:nsl]
                ids[:nsl] = sd["s_sorted"][sd["starts"][nodes] + k]
                blocks[si].append(_wrap16(ids))
        gi_lo = (np.concatenate(blocks[0], axis=1) if blocks[0]
                 else np.zeros((128, 8), np.int16))
        gi_hi = (np.concatenate(blocks[1], axis=1) if blocks[1]
                 else np.zeros((128, 8), np.int16))
        mg_lo = _wrap16(sides[0][c]["slot_of"][n_of_j])
        mg_hi = _wrap16(sides[1][c]["slot_of"][n_of_j] + NSLOT)
        gnode = np.minimum(c * NLOC + n_of_j, N - 1)
        posm = np.ascontiguousarray(
            np.asarray(pos)[gnode].T).astype(np.float16)
        per_core.append({"gi_lo": gi_lo, "gi_hi": gi_hi, "mg_lo": mg_lo,
                         "mg_hi": mg_hi, "posm": posm})
    return per_core, sched


def build_bass(cfg, sched, reps=1, timeline=False, no_cc=False):
    import concourse.bass as bass
    import concourse.bacc as bacc
    import concourse.tile as tile
    from concourse import mybir
    from concourse.masks import make_identity
    import contextlib

    N, NC = cfg["N"], cfg["NC"]
    CIN, COUT = cfg["CIN"], cfg["COUT"]
    LO_LIM = cfg["LO_LIM"]
    NLOC, SLOC, NSLOT, SH, HALFP, RTOT, PV = _derived(cfg)
    HALF = SH * 128
    f32, bf16, i16 = mybir.dt.float32, mybir.dt.float16, mybir.dt.int16
    OP = mybir.AluOpType
    AF = mybir.ActivationFunctionType

    nc = bacc.Bacc(num_devices=(1 if timeline else NC), name="blockconv",
                   dynamic_dma_scratch_size=16384, num_swdge_queues=4)

    xs_in = nc.dram_tensor("xs", [NLOC, CIN], f32, kind="ExternalInput")
    posm_in = nc.dram_tensor("posm", [3, NSLOT], bf16, kind="ExternalInput")
    wt = {}
    for nm, shp in (("W1", [CIN + 3, COUT]), ("b1", [1, COUT]),
                    ("W2", [COUT + 3, COUT]), ("b2", [1, COUT]),
                    ("Wl", [CIN, COUT]), ("bl", [1, COUT]),
                    ("g1", [COUT, 1]), ("be1", [COUT, 1]), ("g2", [COUT, 1]),
                    ("be2", [COUT, 1]), ("gl", [COUT, 1]), ("bel", [COUT, 1])):
        wt[nm] = nc.dram_tensor(nm, shp, f32, kind="ExternalInput")

    Wlo = max(sum(sched[0]), 1) * 8
    Whi = max(sum(sched[1]), 1) * 8
    gi_lo_in = nc.dram_tensor("gi_lo", [128, Wlo], i16, kind="ExternalInput")
    gi_hi_in = nc.dram_tensor("gi_hi", [128, Whi], i16, kind="ExternalInput")
    mg_lo_in = nc.dram_tensor("mg_lo", [128, NSLOT // 16], i16, kind="ExternalInput")
    mg_hi_in = nc.dram_tensor("mg_hi", [128, NSLOT // 16], i16, kind="ExternalInput")

    out_t = nc.dram_tensor("out", [NSLOT, COUT], f32, kind="ExternalOutput")

    shared = "Local" if timeline else "Shared"
    agi = {}
    ago = {}
    for cv in (1, 2):
        for hf in "ab":
            agi[cv, hf] = nc.dram_tensor(f"ag{cv}{hf}_in", [HALFP, COUT], bf16)
            ago[cv, hf] = nc.dram_tensor(f"ag{cv}{hf}_out", [NC, HALFP, COUT],
                                         bf16, addr_space=shared)
    mbuf = nc.dram_tensor("mbuf", [2 * NSLOT, COUT], bf16)
    ar_i = nc.dram_tensor("ar_in", [COUT, 4], f32)
    ar_o = nc.dram_tensor("ar_out", [COUT, 4], f32, addr_space=shared)
    rowbuf = nc.dram_tensor("rowbuf", [6, COUT], f32)
    ar2_i = nc.dram_tensor("ar2_in", [COUT, 2], f32)
    ar2_o = nc.dram_tensor("ar2_out", [COUT, 2], f32, addr_space=shared)
    groups = [list(range(NC))]

    with tile.TileContext(nc) as tc:
        ctx = contextlib.ExitStack()
        with ctx:
            sing = ctx.enter_context(tc.tile_pool(name="sing", bufs=1))
            xp = ctx.enter_context(tc.tile_pool(name="xp", bufs=3))
            pp = ctx.enter_context(tc.tile_pool(name="pp", bufs=2, space="PSUM"))
            pq = ctx.enter_context(tc.tile_pool(name="pq", bufs=2, space="PSUM"))
            pr = ctx.enter_context(tc.tile_pool(name="pr", bufs=1, space="PSUM"))
            cp = ctx.enter_context(tc.tile_pool(name="cp", bufs=4))
            ap_ = ctx.enter_context(tc.tile_pool(name="ap", bufs=1))
            st = ctx.enter_context(tc.tile_pool(name="st", bufs=2))
            sm = ctx.enter_context(tc.tile_pool(name="sm", bufs=2))

            ident = sing.tile([128, 128], f32)
            make_identity(nc, ident)
            identb = sing.tile([128, 128], bf16)
            nc.vector.tensor_copy(out=identb[:], in_=ident[:])
            ones1 = sing.tile([1, 128], f32)
            nc.vector.memset(ones1[:], 1.0)
            ones1v = sing.tile([1, 128], f32)   # valid-partition row mask
            nc.vector.memset(ones1v[:], 1.0)
            if PV < 128:
                nc.vector.memset(ones1v[:, PV:128], 0.0)
            onesp = sing.tile([128, 1], f32)
            nc.vector.memset(onesp[:], 1.0)
            onespb = sing.tile([128, 1], bf16)
            nc.vector.memset(onespb[:], 1.0)
            negbig = sing.tile([128, COUT], bf16)
            nc.vector.memset(negbig[:], BIG_NEG16)
            epsv = sing.tile([COUT, 1], f32)
            nc.vector.memset(epsv[:], EPS)

            W1s = sing.tile([CIN + 3, COUT], f32)
            nc.sync.dma_start(W1s[:], wt["W1"][:])
            W1pb = sing.tile([3, COUT], bf16)
            nc.vector.tensor_copy(out=W1pb[:], in_=W1s[CIN:CIN + 3, :])
            W2af = sing.tile([COUT, COUT], f32)
            nc.sync.dma_start(W2af[:], wt["W2"][0:COUT, :])
            W2ab = sing.tile([COUT, COUT], bf16)
            nc.vector.tensor_copy(out=W2ab[:], in_=W2af[:])
            W2pf = sing.tile([3, COUT], f32)
            nc.sync.dma_start(W2pf[:], wt["W2"][COUT:COUT + 3, :])
            W2pb = sing.tile([3, COUT], bf16)
            nc.vector.tensor_copy(out=W2pb[:], in_=W2pf[:])
            Wlf = sing.tile([CIN, COUT], f32)
            nc.sync.dma_start(Wlf[:], wt["Wl"][:])
            brow = {}
            for nm in ("b1", "b2", "bl"):
                t = sing.tile([1, COUT], f32, tag=f"br_{nm}")
                nc.sync.dma_start(t[:], wt[nm][:])
                brow[nm] = t
            pvec = {}
            for nm in ("g1", "be1", "g2", "be2", "gl", "bel"):
                v = sing.tile([COUT, 1], f32, tag=f"pv_{nm}")
                nc.sync.dma_start(v[:], wt[nm][:])
                pvec[nm] = v

            posmb = sing.tile([3, NSLOT], bf16)
            nc.sync.dma_start(posmb[:], posm_in[:])
            idx_lo = sing.tile([128, Wlo], i16)
            nc.sync.dma_start(idx_lo[:], gi_lo_in[:])
            idx_hi = sing.tile([128, Whi], i16)
            nc.sync.dma_start(idx_hi[:], gi_hi_in[:])
            midx = {}
            for nm, t_ in (("lo", mg_lo_in), ("hi", mg_hi_in)):
                m_ = sing.tile([128, NSLOT // 16], i16, tag=f"mi_{nm}")
                nc.sync.dma_start(m_[:], t_[:])
                midx[nm] = m_

            for _rep in range(reps):
                # -BIG dummy rows HALF..HALF+127 of this core's half-slices
                for cv in (1, 2):
                    for hf in "ab":
                        nc.sync.dma_start(agi[cv, hf][HALF:HALFP, :], negbig[:])

                # ---------- build a1 + skip in slot layout -------------
                def allgather(src, dst):
                    if no_cc:
                        return
                    if timeline:
                        for q in range(_ceil(HALFP, 128)):
                            r0, r1 = q * 128, min((q + 1) * 128, HALFP)
                            t_ = cp.tile([128, COUT], bf16, tag="agb")
                            nc.sync.dma_start(t_[0:r1 - r0], src[r0:r1, :])
                            nc.sync.dma_start(dst[0, r0:r1, :], t_[0:r1 - r0])
                    else:
                        nc.gpsimd.collective_compute(
                            "AllGather", OP.bypass, replica_groups=groups,
                            ins=[src[:]], outs=[dst[:]])

                def half_write(ag, atile, s0):
                    nc.sync.dma_start(
                        ag[0:PV * SH, :].rearrange("(p s) f -> p s f", s=SH),
                        atile[0:PV, s0:s0 + SH, :])

                xbig = sing.tile([128, SLOC, CIN], f32, tag="xbig")
                nc.vector.memset(xbig[:], 0.0)
                nc.sync.dma_start(
                    xbig[0:PV], xs_in[:].rearrange("(p s) c -> p s c", s=SLOC))
                a1 = ap_.tile([128, SLOC, COUT], bf16, tag="big1")
                skt = ap_.tile([128, SLOC, COUT], f32, tag="bigsk")
                for s in range(SLOC):
                    ps = pp.tile([128, 128], f32, tag="pst")
                    nc.tensor.transpose(out=ps[0:CIN, :], in_=xbig[:, s, :], identity=ident[:])
                    xT = xp.tile([CIN, 128], f32, tag="xT")
                    nc.scalar.copy(out=xT[:], in_=ps[0:CIN, :])
                    pb = pq.tile([128, COUT], f32, tag="pout")
                    nc.tensor.matmul(out=pb[:], lhsT=xT[:], rhs=W1s[0:CIN, :], start=True, stop=False)
                    nc.tensor.matmul(out=pb[:], lhsT=posmb[:, s * 128:(s + 1) * 128], rhs=W1pb[:], start=False, stop=False)
                    nc.tensor.matmul(out=pb[:], lhsT=ones1[:], rhs=brow["b1"][:], start=False, stop=True)
                    nc.vector.tensor_copy(out=a1[:, s, :], in_=pb[:])
                    pl = pq.tile([128, COUT], f32, tag="pout")
                    nc.tensor.matmul(out=pl[:], lhsT=xT[:], rhs=Wlf[:], start=True, stop=False)
                    nc.tensor.matmul(out=pl[:], lhsT=ones1v[:], rhs=brow["bl"][:], start=False, stop=True)
                    nc.scalar.copy(out=skt[:, s, :], in_=pl[:])
                    if s == SH - 1:
                        half_write(agi[1, "a"], a1, 0)
                        allgather(agi[1, "a"], ago[1, "a"])

                arst = sing.tile([COUT, 4], f32)

                # PE-chained stats: sums via ones matmul, squares via diag(v^T v)
                def stats_into(vtile, arcols, ones_vec):
                    psum_ = pr.tile([COUT, 1], f32, tag="psum")
                    psq = pr.tile([128, 128], f32, tag="psq")
                    for s in range(SLOC):
                        nc.tensor.matmul(out=psum_[:], lhsT=vtile[:, s, :], rhs=ones_vec[:],
                                         start=(s == 0), stop=(s == SLOC - 1))
                        nc.tensor.matmul(out=psq[:], lhsT=vtile[:, s, :], rhs=vtile[:, s, :],
                                         start=(s == 0), stop=(s == SLOC - 1))
                    sq = sm.tile([128, 128], f32, tag="sqd")
                    nc.vector.tensor_tensor(out=sq[:], in0=psq[:], in1=ident[:], op=OP.mult)
                    nc.vector.tensor_copy(out=arcols[:, 0:1], in_=psum_[:])
                    nc.vector.tensor_reduce(out=arcols[:, 1:2], in_=sq[:],
                                            op=OP.add, axis=mybir.AxisListType.X)

                stats_into(skt, arst[:, 2:4], onesp)


                half_write(agi[1, "b"], a1, SH)
                allgather(agi[1, "b"], ago[1, "b"])

                # ---------------- gather-max passes ----------------
                GMAX = 8   # max 8*128=1024 indices per dma_gather (SWDGE ring cap)

                qrr = [0]   # round-robin over the 4 SWDGE queues

                def gather_chunked(dst3, in_ap, idxt, chunk0, nchunks):
                    a = 0
                    while a < nchunks:
                        b = min(a + GMAX, nchunks)
                        nc.gpsimd.dma_gather(
                            out_ap=dst3[:, a:b, :], in_ap=in_ap,
                            idxs_ap=idxt[:, (chunk0 + a) * 8:(chunk0 + b) * 8],
                            num_idxs=(b - a) * 128, num_idxs_reg=(b - a) * 128,
                            elem_size=COUT, queue_num=qrr[0] % 4)
                        qrr[0] += 1
                        a = b

                def gather_conv(cv, acc_tags):
                    flat_a = ago[cv, "a"][:].rearrange("c n f -> (c n) f")
                    flat_b = ago[cv, "b"][:].rearrange("c n f -> (c n) f")
                    acc = {}
                    for snm, tg in zip(("lo", "hi"), acc_tags):
                        a = ap_.tile([128, SLOC, COUT], bf16, tag=tg)
                        nc.gpsimd.memset(a[:], BIG_NEG16)
                        acc[snm] = a
                    for snm, idxt, flat in (("lo", idx_lo, flat_a),
                                            ("hi", idx_hi, flat_b)):
                        off = 0
                        for k, S in enumerate(sched[0 if snm == "lo" else 1]):
                            stg = st.tile([128, SLOC, COUT], bf16, tag="stage")
                            gather_chunked(stg[:, 0:S, :], flat[:],
                                           idxt, off // 8, S)
                            nc.vector.tensor_tensor(
                                out=acc[snm][:, 0:S, :], in0=acc[snm][:, 0:S, :],
                                in1=stg[:, 0:S, :], op=OP.max)
                            off += 8 * S
                        # merge-buffer write per side (lo's overlaps hi passes)
                        r0 = 0 if snm == "lo" else NSLOT
                        nc.sync.dma_start(
                            mbuf[r0:r0 + NSLOT, :].rearrange("(s p) f -> p s f", p=128),
                            acc[snm][:])
                    g1t = st.tile([128, SLOC, COUT], bf16, tag="stage")
                    gather_chunked(g1t[:], mbuf[:], midx["lo"], 0, SLOC)
                    g2t = st.tile([128, SLOC, COUT], bf16, tag="stage")
                    gather_chunked(g2t[:], mbuf[:], midx["hi"], 0, SLOC)
                    agg = ap_.tile([128, SLOC, COUT], bf16, tag=acc_tags[0])
                    nc.vector.tensor_tensor(out=agg[:], in0=g1t[:], in1=g2t[:], op=OP.max)
                    return agg

                agg1 = gather_conv(1, ("big1", "big2"))

                # v = (agg - pc) * (agg > -1e29), per chunk s
                def v_compute(agg, Wp, vtag, vT=None):
                    v = ap_.tile([128, SLOC, COUT], bf16, tag=vtag)
                    for s in range(SLOC):
                        pc = pq.tile([128, COUT], f32, tag="pout")
                        nc.tensor.matmul(out=pc[:], lhsT=posmb[:, s * 128:(s + 1) * 128],
                                         rhs=Wp[:], start=True, stop=True)
                        msk = cp.tile([128, COUT], bf16, tag="msk")
                        nc.vector.tensor_scalar(out=msk[:], in0=agg[:, s, :], scalar1=-30000.0,
                                                scalar2=None, op0=OP.is_gt)
                        t_ = cp.tile([128, COUT], bf16, tag="tv")
                        nc.vector.tensor_tensor(out=t_[:], in0=agg[:, s, :], in1=pc[:], op=OP.subtract)
                        nc.vector.tensor_tensor(out=v[:, s, :], in0=t_[:], in1=msk[:], op=OP.mult)
                        if vT is not None:
                            pt = pr.tile([128, 128], bf16, tag="psb")
                            nc.tensor.transpose(out=pt[:], in_=v[:, s, :], identity=identb[:])
                            nc.scalar.copy(out=vT[:, s, :], in_=pt[:])
                    return v

                v1T = ap_.tile([128, SLOC, 128], bf16, tag="big3")
                v1 = v_compute(agg1, W1pb, "big2", vT=v1T)
                stats_into(v1, arst[:, 0:2], onespb)
                nc.sync.dma_start(ar_i[:], arst[:])
                if no_cc:
                    pass
                elif timeline:
                    _t = sm.tile([COUT, 4], f32, tag="cc1")
                    nc.sync.dma_start(_t[:], ar_i[:])
                    nc.sync.dma_start(ar_o[:], _t[:])
                else:
                    nc.gpsimd.collective_compute("AllReduce", OP.add, replica_groups=groups,
                                                 ins=[ar_i[:]], outs=[ar_o[:]])
                arres = sing.tile([COUT, 4], f32, tag="arres")
                nc.sync.dma_start(arres[:], ar_o[:])

                # BN params: scale = g * rsqrt(var+eps), shift = be - mean*scale
                def bn_vecs(sum_ap, sq_ap, g_v, be_v, tagp):
                    mean = sm.tile([COUT, 1], f32, tag=f"{tagp}_m")
                    nc.vector.tensor_scalar(out=mean[:], in0=sum_ap, scalar1=1.0 / N,
                                            scalar2=None, op0=OP.mult)
                    ex2 = sm.tile([COUT, 1], f32, tag=f"{tagp}_e")
                    nc.vector.tensor_scalar(out=ex2[:], in0=sq_ap, scalar1=1.0 / N,
                                            scalar2=None, op0=OP.mult)
                    m2 = sm.tile([COUT, 1], f32, tag=f"{tagp}_m2")
                    nc.vector.tensor_tensor(out=m2[:], in0=mean[:], in1=mean[:], op=OP.mult)
                    var = sm.tile([COUT, 1], f32, tag=f"{tagp}_v")
                    nc.vector.tensor_tensor(out=var[:], in0=ex2[:], in1=m2[:], op=OP.subtract)
                    sd = sm.tile([COUT, 1], f32, tag=f"{tagp}_sd")
                    nc.scalar.activation(out=sd[:], in_=var[:], func=AF.Sqrt, bias=epsv[:], scale=1.0)
                    rstd = sm.tile([COUT, 1], f32, tag=f"{tagp}_r")
                    nc.vector.reciprocal(out=rstd[:], in_=sd[:])
                    ssh = sm.tile([COUT, 2], f32, tag=f"{tagp}_ssh")
                    nc.vector.tensor_tensor(out=ssh[:, 0:1], in0=rstd[:], in1=g_v[:], op=OP.mult)
                    ms = sm.tile([COUT, 1], f32, tag=f"{tagp}_ms")
                    nc.vector.tensor_tensor(out=ms[:], in0=mean[:], in1=ssh[:, 0:1], op=OP.mult)
                    nc.vector.tensor_tensor(out=ssh[:, 1:2], in0=be_v[:], in1=ms[:], op=OP.subtract)
                    return ssh

                def bn_rows(ssh, tagp):
                    # transpose [COUT,2] -> [2, COUT] rows (scale row 0, shift row 1)
                    prow = pr.tile([2, COUT], f32, tag="prow")
                    nc.tensor.transpose(out=prow[:], in_=ssh[:], identity=ident[:])
                    rows = sing.tile([2, COUT], f32, tag=f"{tagp}_rows")
                    nc.vector.tensor_copy(out=rows[:], in_=prow[:])
                    slot = {"bn1": 0, "bnl": 2, "bn2": 4}[tagp]
                    nc.sync.dma_start(rowbuf[slot:slot + 2, :], rows[:])
                    bc = sing.tile([128, 2, COUT], f32, tag=f"{tagp}_bc")
                    rap = rowbuf[slot:slot + 2, :]
                    nc.sync.dma_start(bc[:], bass.AP(tensor=rap.tensor, offset=rap.offset,
                                                     ap=[[0, 128]] + list(rap.ap)))
                    return bc

                ssh1 = bn_vecs(arres[:, 0:1], arres[:, 1:2], pvec["g1"], pvec["be1"], "bn1")
                sshl = bn_vecs(arres[:, 2:3], arres[:, 3:4], pvec["gl"], pvec["bel"], "bnl")
                rowsl = bn_rows(sshl, "bnl")

                # h1T = relu(v1T*scale1 + shift1): one Act op in transposed
                # (feature-major) layout, using the v1T built during the AR
                nc.scalar.activation(out=v1T[:].rearrange("f s n -> f (s n)"),
                                     in_=v1T[:].rearrange("f s n -> f (s n)"),
                                     func=AF.Relu, bias=ssh1[:, 1:2], scale=ssh1[:, 0:1])

                # ---------- a2 = [h1|pos]@W2 + b2 (slot layout) --------
                a2 = ap_.tile([128, SLOC, COUT], bf16, tag="big1")
                for s in range(SLOC):
                    pb = pq.tile([128, COUT], f32, tag="pout")
                    nc.tensor.matmul(out=pb[:], lhsT=v1T[:, s, :], rhs=W2ab[:], start=True, stop=False)
                    nc.tensor.matmul(out=pb[:], lhsT=posmb[:, s * 128:(s + 1) * 128], rhs=W2pb[:], start=False, stop=False)
                    nc.tensor.matmul(out=pb[:], lhsT=ones1[:], rhs=brow["b2"][:], start=False, stop=True)
                    nc.vector.tensor_copy(out=a2[:, s, :], in_=pb[:])
                    if s == SH - 1:
                        half_write(agi[2, "a"], a2, 0)
                        allgather(agi[2, "a"], ago[2, "a"])
                half_write(agi[2, "b"], a2, SH)
                allgather(agi[2, "b"], ago[2, "b"])

                agg2 = gather_conv(2, ("big1", "big2"))
                v2 = v_compute(agg2, W2pb, "big2")

                arst2 = sing.tile([COUT, 2], f32, tag="arst2")
                stats_into(v2, arst2[:], onespb)
                nc.sync.dma_start(ar2_i[:], arst2[:])
                if no_cc:
                    pass
                elif timeline:
                    _t = sm.tile([COUT, 2], f32, tag="cc2")
                    nc.sync.dma_start(_t[:], ar2_i[:])
                    nc.sync.dma_start(ar2_o[:], _t[:])
                else:
                    nc.gpsimd.collective_compute("AllReduce", OP.add, replica_groups=groups,
                                                 ins=[ar2_i[:]], outs=[ar2_o[:]])
                arres2 = sing.tile([COUT, 2], f32, tag="arres2")
                nc.sync.dma_start(arres2[:], ar2_o[:])
                rows2 = bn_rows(bn_vecs(arres2[:, 0:1], arres2[:, 1:2], pvec["g2"], pvec["be2"], "bn2"), "bn2")

                # final = relu(bn2(v2) + bnl(skip)); skt transformed in place
                fin = ap_.tile([128, SLOC, COUT], f32, tag="bigf")
                nc.vector.tensor_tensor(out=fin[:], in0=v2[:],
                                        in1=rows2[:, 0:1, :].to_broadcast([128, SLOC, COUT]), op=OP.mult)
                nc.vector.tensor_tensor(out=fin[:], in0=fin[:],
                                        in1=rows2[:, 1:2, :].to_broadcast([128, SLOC, COUT]), op=OP.add)
                nc.vector.tensor_tensor(out=skt[:], in0=skt[:],
                                        in1=rowsl[:, 0:1, :].to_broadcast([128, SLOC, COUT]), op=OP.mult)
                nc.vector.tensor_tensor(out=skt[:], in0=skt[:],
                                        in1=rowsl[:, 1:2, :].to_broadcast([128, SLOC, COUT]), op=OP.add)
                nc.vector.tensor_tensor(out=fin[:], in0=fin[:], in1=skt[:], op=OP.add)
                nc.vector.tensor_scalar(out=fin[:], in0=fin[:], scalar1=0.0, scalar2=None, op0=OP.max)
                nc.sync.dma_start(out_t[:].rearrange("(p s) f -> p s f", p=128), fin[:])

    nc.compile()
    return nc


def make_in_maps(inputs, cfg, per_core):
    N, NC, CIN = cfg["N"], cfg["NC"], cfg["CIN"]
    NLOC = N // NC
    x = np.ascontiguousarray(np.asarray(inputs["x"], np.float32))
    shared = dict(
        W1=np.asarray(inputs["W1"], np.float32),
        b1=np.asarray(inputs["b1"], np.float32).reshape(1, -1),
        W2=np.asarray(inputs["W2"], np.float32),
        b2=np.asarray(inputs["b2"], np.float32).reshape(1, -1),
        Wl=np.asarray(inputs["Wl"], np.float32),
        bl=np.asarray(inputs["bl"], np.float32).reshape(1, -1),
        g1=np.asarray(inputs["g1"], np.float32).reshape(-1, 1),
        be1=np.asarray(inputs["be1"], np.float32).reshape(-1, 1),
        g2=np.asarray(inputs["g2"], np.float32).reshape(-1, 1),
        be2=np.asarray(inputs["be2"], np.float32).reshape(-1, 1),
        gl=np.asarray(inputs["gl"], np.float32).reshape(-1, 1),
        bel=np.asarray(inputs["bel"], np.float32).reshape(-1, 1),
    )
    in_maps = []
    for c in range(NC):
        m = dict(shared)
        m["xs"] = np.ascontiguousarray(x[c * NLOC:(c + 1) * NLOC])
        pc = per_core[c]
        for k in ("gi_lo", "gi_hi", "mg_lo", "mg_hi", "posm"):
            m[k] = pc[k]
        in_maps.append(m)
    return in_maps


_CACHE = {}


def run(inputs, cfg, use_sim=False, trace=False):
    per_core, sched = host_prep(inputs["edge_index"], inputs["pos"], cfg)
    key = (cfg["N"], tuple(sched[0]), tuple(sched[1]))
    if key not in _CACHE:
        _CACHE[key] = build_bass(cfg, sched)
    nc = _CACHE[key]
    in_maps = make_in_maps(inputs, cfg, per_core)
    NC = cfg["NC"]
    NLOC = cfg["N"] // NC
    if use_sim:
        from concourse.bass_interp import MultiCoreSim
        sim = MultiCoreSim(nc, num_cores=NC, require_finite=False, require_nnan=False)
        for c in range(NC):
            for k, v in in_maps[c].items():
                sim.cores[c].tensor(k)[:] = v
        sim.simulate(check_with_hw=False)
        outs = [np.array(sim.cores[c].tensor("out")) for c in range(NC)]
        res = None
    else:
        from concourse.bass_utils import run_bass_kernel_spmd
        res = run_bass_kernel_spmd(nc, in_maps, core_ids=list(range(NC)), trace=trace)
        outs = [res.results[c]["out"] for c in range(NC)]
    full = np.concatenate([o[:NLOC] for o in outs], axis=0)
    return full, res


def kernel(**inputs):
    out, _ = run(inputs, FULL_CFG, use_sim=False)
    return out
